# revision 1
# baseline (speedup 1.0000x reference)
"""Causal MHA with RoPE on 8 TRN2 NeuronCores.

Sharding: data-parallel over batch (2) x tensor-parallel over heads (4 groups
of 4 heads) = 8 cores. Core c handles batch c//4, head group c%4.
Each core computes its 4 heads' attention and a partial output projection
(Wo sharded row-wise); host sums the 4 partials per batch.

Per-core device algorithm (all matmuls bf16 inputs, fp32 PSUM accumulate):
  - QK^T projection: qkT[dk, s] = (Wqk rows).T-contracted with xT
    (host-transposed, bf16), RoPE applied on the [dk(partition), s] layout
    via cos/sin tables and a stream_shuffle partition pair-swap
  - scores^T[k, q] = K^T.T-free @ Q^T per head (K=64 contraction, two heads
    packed in row groups 0-1 / 2-3 of the PE array)
  - probsT = exp(scores/8) straight from PSUM (no max subtraction; scores are
    N(0,1)-scaled so exp never overflows), causal tri-mask on diagonal tiles
  - PV is FLIPPED: attn[q, dk+1] = probsT.T @ [V|1] per 128-q-block (N=65
    moving rows, full 128-wide contraction), accumulated over k-blocks in a
    PSUM bank; the ones column makes the softmax denominator a per-partition
    scalar, so normalization is a reciprocal + broadcast multiply; a PE
    transpose then restores the [feature, q] layout for the out projection
  - partial out = attnT.T-contracted with WoT chunks over both head pairs

Schedule: the kernel is one software-pipelined stream ordered for the
(in-order) engines. The attention kb-loop is ACT(exp)-paced, so all other
PE work - next x-tile's QK/V projection chunks, attention transposes, and
output projections - is queued and pumped into the kb-steps' idle slots.
Pairs are processed in an ACT-leveled order across q-tiles, and output
projections are deferred to the ACT-heaviest stretch. PSUM accumulation
exploits the HW behavior that a start=True matmul zeroes its whole bank.
"""
import sys
import os

for _p in ("/opt/trn_rl_repo", "/root/.axon_site/_ro/trn_rl_repo"):
    if os.path.isdir(_p) and _p not in sys.path:
        sys.path.insert(0, _p)

import numpy as np

import concourse.mybir as mybir
import concourse.tile as tile
from concourse import bacc
from concourse.bass_utils import run_bass_kernel_spmd

F32 = mybir.dt.float32
F32R = mybir.dt.float32r
BF16 = mybir.dt.bfloat16
AF = mybir.ActivationFunctionType
MULT = mybir.AluOpType.mult
ADD = mybir.AluOpType.add
DIV = mybir.AluOpType.divide

B, S, D = 2, 2048, 1024
H, DK = 16, 64
THETA = 10000.0
NCORES = 8
GROUPS = 4          # head groups (tensor parallel)
GH = H // GROUPS    # heads per group = 4
GF = GH * DK        # features per group = 256
SWAP_MASK = [i ^ 1 for i in range(32)]
KVER = 49  # bump on any kernel change: busts the HLO-shape-keyed NEFF cache

_CACHED = {}


def _build_nc(iters=1):
    _iters = iters
    nc = bacc.Bacc("TRN2", target_bir_lowering=False, debug=False, num_devices=NCORES)
    xT = nc.dram_tensor("xT", [D, S], BF16, kind="ExternalInput").ap()
    wqkT = nc.dram_tensor("wqkT", [D, 2 * GF], BF16, kind="ExternalInput").ap()
    wvT = nc.dram_tensor("wvT", [D, GF], BF16, kind="ExternalInput").ap()
    woT = nc.dram_tensor("woT", [GF, D], BF16, kind="ExternalInput").ap()
    cosf = nc.dram_tensor("cosf", [128, S], BF16, kind="ExternalInput").ap()
    sins = nc.dram_tensor("sins", [128, S], BF16, kind="ExternalInput").ap()
    tri = nc.dram_tensor("tri", [128, 128], BF16, kind="ExternalInput").ap()
    ident = nc.dram_tensor("ident", [128, 128], BF16, kind="ExternalInput").ap()
    onesc = nc.dram_tensor("onesc", [128, 1], F32R, kind="ExternalInput").ap()
    # unused input whose shape encodes the kernel version: the neuron compile
    # cache keys on HLO structure only, so two kernels with identical I/O
    # shapes would otherwise collide.
    nc.dram_tensor("cachebust", [iters, KVER], F32, kind="ExternalInput")
    out = nc.dram_tensor("out", [S, D], BF16, kind="ExternalOutput").ap()

    SB = S // 512  # 4 q-tiles of 512
    KB = S // 128  # 16 k-blocks of 128

    with tile.TileContext(nc) as tc:
        with tc.tile_pool(name="const", bufs=1) as cpool, \
             tc.tile_pool(name="big", bufs=1) as bpool, \
             tc.tile_pool(name="work", bufs=2) as wpool, \
             tc.tile_pool(name="asb", bufs=3) as apool, \
             tc.tile_pool(name="probs", bufs=4) as ppool, \
             tc.tile_pool(name="obuf", bufs=4) as opool, \
             tc.tile_pool(name="psum", bufs=1, space="PSUM") as psum:

            # ---- loads, ordered by first use on the single HWDGE queue ----
            wqk_sb = cpool.tile([128, 8, 2 * GF], BF16, tag="wqk")
            wv_sb = cpool.tile([128, 8, GF], BF16, tag="wv")
            wo_sb = cpool.tile([128, 2, D], BF16, tag="wo")
            cos_sb = cpool.tile([128, S], BF16, tag="cos")
            sin_sb = cpool.tile([128, S], BF16, tag="sin")
            tri_sb = cpool.tile([128, 128], BF16, tag="tri")
            ident_sb = cpool.tile([128, 128], BF16, tag="ident")
            onesc_sb = cpool.tile([128, 1], F32R, tag="onesc")
            xt_all = cpool.tile([128, 8, S], BF16, tag="xt")

            def xt_load(tsl, nway=2, eng=None):
                # x feature-chunks dc packed on partitions, batched DMAs
                w = 8 // nway
                for i in range(nway):
                    (eng or nc.sync).dma_start(
                        xt_all[:, i * w:(i + 1) * w, tsl],
                        xT[i * w * 128:(i + 1) * w * 128, tsl].rearrange(
                            "(dc p) s -> p dc s", p=128))

            def wqk_load(c):
                nc.sync.dma_start(
                    wqk_sb[:, :, c * 128:(c + 1) * 128],
                    wqkT[:, c * 128:(c + 1) * 128].rearrange(
                        "(dc p) n -> p dc n", p=128))

            # first tile's inputs go down two DMA queues in parallel:
            # weights on the SP queue, x chunks on the DVE queue
            for halfc in range(2):
                nc.sync.dma_start(
                    wqk_sb[:, halfc * 4:(halfc + 1) * 4, 0:128],
                    wqkT[halfc * 512:(halfc + 1) * 512, 0:128].rearrange(
                        "(dc p) n -> p dc n", p=128))
            xt_load(slice(0, 512), nway=4)
            nc.sync.dma_start(wv_sb[:], wvT.rearrange("(dc p) n -> p dc n", p=128))
            wqk_load(2)
            nc.sync.dma_start(onesc_sb[:], onesc)
            nc.sync.dma_start(cos_sb[:, 0:1024], cosf[:, 0:1024])
            nc.sync.dma_start(sin_sb[:, 0:1024], sins[:, 0:1024])
            wqk_load(1)
            wqk_load(3)
            nc.sync.dma_start(tri_sb[:], tri)
            xt_load(slice(512, 1024), nway=4)
            nc.sync.dma_start(ident_sb[:], ident)
            nc.sync.dma_start(cos_sb[:, 1024:S], cosf[:, 1024:S])
            nc.sync.dma_start(sin_sb[:, 1024:S], sins[:, 1024:S])
            nc.sync.dma_start(wo_sb[:], woT.rearrange("(fc p) n -> p fc n", p=128))
            xt_load(slice(1024, S))

            warm = cpool.tile([1, 1], F32, tag="warm")
            nc.scalar.activation(warm[:], onesc_sb[0:1, 0:1], AF.Exp, scale=1.0)
            # Warm-up matmuls during the initial DMA wait: the PE runs at
            # half rate until it has been busy 3us, so burn the p-state ramp
            # on a zero row instead of the first real projections.
            zrow = cpool.tile([1, 512], BF16, tag="zrow")
            nc.vector.memset(zrow[:], 0.0)
            for w in range(14):
                pw = psum.tile([128, 512], F32, tag="sc", bufs=2,
                               name=f"warmmm{w}")
                nc.tensor.matmul(pw[:], zrow[0:1, 0:128], zrow[0:1, :],
                                 start=True, stop=True)

            # ---- kernel body ----
            # Projections (phase 1) for x-tile t+1 are interleaved INTO the
            # attention kb-loop over qt=t: attention is ACT(exp)-bound, so the
            # PE fills its idle slots with the next tile's QK/V projections.
            for _it in range(iters):
              qkT = bpool.tile([128, 4, S], BF16, tag="qkT", name=f"qkT{_it}")
              vt = bpool.tile([128, KB, GH, DK + 1], BF16, tag="vt", name=f"vt{_it}")
              nc.vector.tensor_copy(
                  vt[:, :, :, DK:DK + 1],
                  onesc_sb[:, None, None, :].to_broadcast([128, KB, GH, 1]))

              def proj_qk_half(t, c, half, ps):
                  for dc in range(4 * half, 4 * half + 4):
                      nc.tensor.matmul(
                          ps[:], wqk_sb[:, dc, c * 128:(c + 1) * 128],
                          xt_all[:, dc, t * 512:(t + 1) * 512],
                          start=(dc == 0), stop=(dc == 7))

              def proj_qk(t, c, fast=False, half=None):
                  # QK projection chunk: 128 features (head pair c of Q|K),
                  # 512 seq positions, full D contraction; then RoPE.
                  ps = psum.tile([128, 512], F32, tag="sc", bufs=2)
                  proj_qk_half(t, c, 0, ps)
                  if half is not None:
                      # second half (+ RoPE) deferred as the next filler unit
                      half.appendleft(lambda: proj_qk_rope(t, c, ps, fast))
                      return
                  proj_qk_rope(t, c, ps, fast)

              def proj_qk_rope(t, c, ps, fast=False):
                  proj_qk_half(t, c, 1, ps)
                  tsl = slice(t * 512, (t + 1) * 512)
                  # rope: qkT = ps*cos + swap(ps*sins), all on DVE (Pool is
                  # reserved for the latency-critical causal masks)
                  if fast:
                      # prologue chunks: pre-round ps to bf16 on the (idle)
                      # ACT engine so the DVE multiplies run in 2x mode
                      psb = wpool.tile([128, 512], BF16, tag="psb")
                      nc.scalar.copy(psb[:], ps[:])
                      src = psb
                  else:
                      src = ps
                  tmp = wpool.tile([128, 512], BF16, tag="ropetmp")
                  nc.vector.tensor_tensor(tmp[:], src[:], sin_sb[:, tsl], MULT)
                  tmp2 = wpool.tile([128, 512], BF16, tag="ropetmp2")
                  nc.vector.stream_shuffle(tmp2[:], tmp[:], SWAP_MASK)
                  nc.vector.tensor_tensor(qkT[:, c, tsl], src[:], cos_sb[:, tsl], MULT)
                  nc.gpsimd.tensor_tensor(qkT[:, c, tsl], qkT[:, c, tsl], tmp2[:], ADD)

              def proj_v(sb_i, on_act=False):
                  psv = psum.tile([128, GF], F32, tag="sc", bufs=2)
                  for dc in range(8):
                      nc.tensor.matmul(
                          psv[:], xt_all[:, dc, sb_i * 128:(sb_i + 1) * 128],
                          wv_sb[:, dc, :], start=(dc == 0), stop=(dc == 7))
                  if on_act:
                      nc.scalar.copy(vt[:, sb_i, :, 0:DK],
                                     psv[:].rearrange("p (h d) -> p h d", h=GH))
                  else:
                      nc.vector.tensor_copy(
                          vt[:, sb_i, :, 0:DK],
                          psv[:].rearrange("p (h d) -> p h d", h=GH))

              # Deferred-PE-work queue: projection chunks for tile t+1,
              # transposes of the previous pair, and the previous qt's output
              # projection all get pumped into the attention kb-loop so the
              # (in-order) PE never sits behind a dependency-stalled
              # instruction for long.
              from collections import deque
              fill_q = deque()   # prompt PE work (projections, transposes)
              late_q = deque()   # output projections, deferred to late qts
                                 # where attention has an ACT-vs-PE deficit

              def pump(n=1, late_ok=False):
                  for _ in range(n):
                      if fill_q:
                          fill_q.popleft()()
                      elif late_ok and late_q:
                          late_q.popleft()()

              def transpose_unit(gq, pair, att_sb, qb):
                  def run():
                      tps = psum.tile([128, 128], BF16, tag="sc", bufs=2,
                                      name=f"tps{gq}_{pair}_{_it}")
                      nc.tensor.transpose(tps[:], att_sb[:, qb, :], ident_sb[:])
                      nc.vector.tensor_copy(
                          attnT[pair][:, gq * 128:(gq + 1) * 128], tps[:])
                  return run

              def oproj_units(qb, split_copy=False):
                  # output projection for one 128-q-block, as two filler units
                  # (one per 512-wide n-half; the second issues the DMA)
                  st = {}

                  def run_nh(nh):
                      qsl = slice(qb * 128, (qb + 1) * 128)
                      if nh == 0:
                          st["osb"] = opool.tile([128, D], BF16, tag="osb",
                                                 name=f"osb{qb}_{_it}")
                      osb = st["osb"]
                      nsl = slice(nh * 512, (nh + 1) * 512)
                      pso = psum.tile([128, 512], F32, tag="sc", bufs=2)
                      nc.tensor.matmul(pso[:], attnT[0][:, qsl],
                                       wo_sb[:, 0, nsl], start=True, stop=False)
                      nc.tensor.matmul(pso[:], attnT[1][:, qsl],
                                       wo_sb[:, 1, nsl], start=False, stop=True)
                      if split_copy and nh == 1:
                          nc.scalar.copy(osb[:, nsl], pso[:])
                      else:
                          nc.vector.tensor_copy(osb[:, nsl], pso[:])
                      if split_copy:
                          # last qt: one DMA per n-half so the final transfer
                          # is half-size
                          nc.sync.dma_start(out[qsl, nsl], osb[:, nsl])
                      elif nh == 1:
                          nc.sync.dma_start(out[qsl, :], osb[:])

                  return [lambda: run_nh(0), lambda: run_nh(1)]

              # ---- attention (PV-flipped), deferred work interleaved ----
              # PV: attn[q, dk] = probsT.T @ [V|1] per 128-q-block: N=65 moving
              # rows instead of N=512, fully using the 128-wide K (k-positions)
              # and M (q) dims of the PE array. Softmax sums land in column 64
              # as per-partition scalars -> normalization via Pool broadcast
              # multiply, then a PE transpose restores [f, q] layout for the
              # output projection.
              attnT = [bpool.tile([128, S], BF16, tag=f"attnT{p}",
                                  name=f"attnT{p}_{_it}") for p in range(2)]

              # tile t=0: pair-0's needs (Q01, K01, V) up front; Q23/K23 queued.
              # fast=True / on_act=True shift prologue elementwise work onto
              # the idle ACT engine to shorten the first-attention latency.
              proj_qk(0, 0, fast=True)
              proj_qk(0, 2, fast=True)
              for s in range(4):
                  proj_v(s, on_act=True)
              fill_q.append(lambda: proj_qk(0, 1, fast=True))
              fill_q.append(lambda: proj_qk(0, 3, fast=True))

              def enqueue_proj(t):
                  for c in range(4):
                      fill_q.append(lambda c=c: proj_qk(t, c, fast=True, half=fill_q))
                  for s in range(4 * t, 4 * t + 4):
                      fill_q.append(lambda s=s: proj_v(s))

              # Pair schedule, leveled so the ACT-heavy qt=3 pairs sit next to
              # filler-rich regions instead of clustering at the end. proj[t]
              # is enqueued just before the first pair that leaves enough
              # steps to drain it ahead of its consumer.
              SCHED = [(0, 0, 1), (0, 1, None), (1, 0, 2), (1, 1, None),
                       (2, 0, 3), (3, 0, None), (2, 1, None), (3, 1, None)]
              done_pairs = set()

              def do_pair(qt, pair, late_ok):
                  nkb = 4 * qt + 4
                  nsteps = nkb
                  step = 0
                  qs, ks = pair, 2 + pair
                  last = qt == SB - 1 and pair == 1
                  # Interleaved accumulation GROUPS in one PSUM bank are
                  # broken on HW: start=True zeroes the WHOLE bank (HW
                  # verified). Exploit that: the first PV write of each
                  # h-bank (kb=0, qb=0) runs with start=True to zero the
                  # bank, everything else accumulates with start=False.
                  attps = [psum.tile([128, 4, DK + 1], F32, tag="att",
                                     bufs=2, name=f"attps{h}_{qt}_{pair}_{_it}")
                           for h in range(2)]

                  def pv(kb):
                      for qb in range(max(kb - 4 * qt, 0), 4):
                          gq = 4 * qt + qb
                          for h in range(2):
                              nc.tensor.matmul(
                                  attps[h][:, qb, :],
                                  pab_ring[kb % 32][:, h, qb * 128:(qb + 1) * 128],
                                  vt[:, kb, 2 * pair + h, :],
                                  start=(kb == 0 and qb == 0),
                                  stop=(kb == gq),
                                  skip_group_check=True)

                  att_sb = apool.tile([128, 4, 128], BF16, tag="attsb")
                  rsum = wpool.tile([128, 4, 2], F32, tag="rsum")

                  def normalize(qb):
                      # DVE reciprocal of the col-64 sums + DVE broadcast
                      # multiply (gpsimd cannot read PSUM). Per-q-block only
                      # for the very last pair (to pipeline the tail);
                      # batched per-pair otherwise to keep the DVE
                      # instruction count down. Transposes go to the FRONT of
                      # the queue: they are small, release the shared "sc"
                      # PSUM ring fast, and feed the output projection.
                      if not last:
                          if qb < 3:
                              return
                          qsl3, nq = slice(0, 4), 4
                      else:
                          qsl3, nq = slice(qb, qb + 1), 1
                      for h in range(2):
                          nc.vector.reciprocal(rsum[:, qsl3, h:h + 1],
                                               attps[h][:, qsl3, DK:DK + 1])
                          nc.vector.tensor_tensor(
                              att_sb[:, qsl3, h * 64:(h + 1) * 64],
                              attps[h][:, qsl3, 0:DK],
                              rsum[:, qsl3, h:h + 1].to_broadcast(
                                  [128, nq, DK]), MULT)
                      if last:
                          # last pair: output projection chases each q-block
                          # (PE transpose: shorter latency than the DMA XBAR)
                          for u in reversed(oproj_units(4 * qt + qb,
                                                        split_copy=True)):
                              fill_q.appendleft(u)
                          fill_q.appendleft(
                              transpose_unit(4 * qt + qb, pair, att_sb, qb))
                      else:
                          # engine-free transpose via the DMA XBAR: frees PE
                          # rows, the DVE copy, and the "sc" PSUM ring
                          for b in range(4):
                              gq = 4 * qt + b
                              nc.sync.dma_start_transpose(
                                  attnT[pair][:, gq * 128:(gq + 1) * 128],
                                  att_sb[:, b, :])

                  pab_ring = {}
                  for kb in range(nkb):
                      lam = max(kb - 4 * qt, 0) * 128
                      qsl = slice(qt * 512 + lam, (qt + 1) * 512)
                      ksl = slice(kb * 128, (kb + 1) * 128)
                      ss = psum.tile([128, 2, 512], F32, tag="sc2", bufs=2)
                      nc.tensor.matmul(ss[:, 0, lam:512], qkT[0:64, ks, ksl],
                                       qkT[0:64, qs, qsl], start=True, stop=True)
                      nc.tensor.matmul(ss[:, 1, lam:512], qkT[64:128, ks, ksl],
                                       qkT[64:128, qs, qsl], start=True, stop=True)
                      pab = ppool.tile([128, 2, 512], BF16, tag="probs", bufs=32)
                      pab_ring[kb % 32] = pab
                      nc.scalar.activation(pab[:, :, lam:512], ss[:, :, lam:512], AF.Exp, scale=0.125)
                      if kb >= 4 * qt:  # diagonal block: causal tri mask
                          # on DVE: all-bf16 SBUF operands hit 2x mode,
                          # shortening the exp->mask->PV diagonal chain
                          dsl = slice(lam, lam + 128)
                          nc.vector.tensor_tensor(
                              pab[:, :, dsl], pab[:, :, dsl],
                              tri_sb[:, None, :].to_broadcast([128, 2, 128]), MULT)
                      # drain queued PE work evenly across this pair's steps;
                      # late (output-projection) work backfills ACT-bound pairs
                      n = -(-len(fill_q) // (nsteps - step)) if fill_q else 0
                      if kb >= 4 * qt and late_ok:
                          n = max(n, 2)  # diag steps absorb more filler
                      if late_ok and (step % 2 == 0 or kb >= 4 * qt):
                          n = max(n, 1)
                      pump(n, late_ok=late_ok)
                      step += 1
                      # software pipeline: PV for the previous kb runs after
                      # this kb's scores are already in flight
                      if kb > 0:
                          pv(kb - 1)
                          if kb - 1 >= 4 * qt:  # that region just stopped
                              normalize(kb - 1 - 4 * qt)
                  pv(nkb - 1)
                  normalize(3)
                  done_pairs.add((qt, pair))
                  if (qt, 1 - pair) in done_pairs and not last:
                      for qb in range(4 * qt, 4 * qt + 4):
                          late_q.extend(oproj_units(qb))

              for i, (qt, pair, tload) in enumerate(SCHED):
                  if tload is not None:
                      enqueue_proj(tload)
                  do_pair(qt, pair, late_ok=(i >= len(SCHED) - 3))
              while late_q:
                  late_q.popleft()()
              while fill_q:
                  fill_q.popleft()()

    nc.compile()
    return nc


def _host_tables(token_positions):
    pos = np.asarray(token_positions, dtype=np.float32)  # [S]
    half = DK // 2
    freq = THETA ** (-np.arange(0, DK, 2, dtype=np.float32) / DK)  # [32]
    # per-partition tables on [dk(128 = 2 heads of 64), s]
    f64 = np.repeat(freq, 2)          # [64] freq per feature index
    ang64 = pos[None, :] * f64[:, None]  # [64, S]
    cos64 = np.cos(ang64)
    sin64 = np.sin(ang64)
    sign = np.where(np.arange(DK) % 2 == 0, 1.0, -1.0).astype(np.float32)  # +s even, -s odd
    sins64 = sin64 * sign[:, None]
    from ml_dtypes import bfloat16 as bf16
    cosf = np.concatenate([cos64, cos64], axis=0).astype(bf16)   # [128, S]
    sins = np.concatenate([sins64, sins64], axis=0).astype(bf16)  # [128, S]
    return cosf, sins


def kernel(x, Wq, Wk, Wv, Wo, token_positions):
    from ml_dtypes import bfloat16 as bf16
    x = np.asarray(x, dtype=np.float32)
    Wq = np.asarray(Wq, dtype=np.float32)
    Wk = np.asarray(Wk, dtype=np.float32)
    Wv = np.asarray(Wv, dtype=np.float32)
    Wo = np.asarray(Wo, dtype=np.float32)

    if "nc" not in _CACHED:
        _CACHED["nc"] = _build_nc(iters=int(os.environ.get("BENCH_ITERS", "1")))
    nc = _CACHED["nc"]

    cosf, sins = _host_tables(token_positions)
    tri = np.triu(np.ones((128, 128), dtype=bf16))  # tri[k, j] = 1 if j >= k
    ident = np.eye(128, dtype=bf16)
    onesc = np.ones((128, 1), dtype=np.float32)

    xT = [np.ascontiguousarray(x[b].T).astype(bf16) for b in range(B)]  # [D, S]
    in_maps = []
    for c in range(NCORES):
        b, g = c // GROUPS, c % GROUPS
        R = slice(g * GF, (g + 1) * GF)
        wqkT = np.ascontiguousarray(
            np.concatenate([Wq[R].T, Wk[R].T], axis=1)).astype(bf16)  # [D, 512]
        wvT = np.ascontiguousarray(Wv[R].T).astype(bf16)              # [D, 256]
        woT = np.ascontiguousarray(Wo[:, R].T).astype(bf16)           # [256, D]
        in_maps.append({
            "xT": xT[b], "wqkT": wqkT, "wvT": wvT, "woT": woT,
            "cosf": cosf, "sins": sins, "tri": tri, "ident": ident, "onesc": onesc,
            "cachebust": np.zeros((int(os.environ.get("BENCH_ITERS", "1")), KVER), dtype=np.float32),
        })

    try:
        res = run_bass_kernel_spmd(nc, in_maps, core_ids=list(range(NCORES)))
    except Exception:
        # transient NRT_EXEC_UNIT_UNRECOVERABLE flakes recover on retry
        import time as _time
        _time.sleep(2.0)
        res = run_bass_kernel_spmd(nc, in_maps, core_ids=list(range(NCORES)))
    _CACHED["last_results"] = res
    outs = [np.asarray(r["out"], dtype=np.float32) for r in res.results]  # [S, D]
    full = np.empty((B, S, D), dtype=np.float32)
    for b in range(B):
        full[b] = sum(outs[b * GROUPS + g] for g in range(GROUPS))
    return full



# revision 45
# speedup vs baseline: 1.0034x; 1.0034x over previous
"""Causal MHA with RoPE on 8 TRN2 NeuronCores.

Sharding: data-parallel over batch (2) x tensor-parallel over heads (4 groups
of 4 heads) = 8 cores. Core c handles batch c//4, head group c%4.
Each core computes its 4 heads' attention and a partial output projection
(Wo sharded row-wise); host sums the 4 partials per batch.

Per-core device algorithm (all matmuls bf16 inputs, fp32 PSUM accumulate):
  - QK^T projection: qkT[dk, s] = (Wqk rows).T-contracted with xT
    (host-transposed, bf16), RoPE applied on the [dk(partition), s] layout
    via cos/sin tables and a stream_shuffle partition pair-swap
  - scores^T[k, q] = K^T.T-free @ Q^T per head (K=64 contraction, two heads
    packed in row groups 0-1 / 2-3 of the PE array)
  - probsT = exp(scores/8) straight from PSUM (no max subtraction; scores are
    N(0,1)-scaled so exp never overflows), causal tri-mask on diagonal tiles
  - PV is FLIPPED: attn[q, dk+1] = probsT.T @ [V|1] per 128-q-block (N=65
    moving rows, full 128-wide contraction), accumulated over k-blocks in a
    PSUM bank; the ones column makes the softmax denominator a per-partition
    scalar, so normalization is a reciprocal + broadcast multiply; a PE
    transpose then restores the [feature, q] layout for the out projection
  - partial out = attnT.T-contracted with WoT chunks over both head pairs

Schedule: the kernel is one software-pipelined stream ordered for the
(in-order) engines. The attention kb-loop is ACT(exp)-paced, so all other
PE work - next x-tile's QK/V projection chunks, attention transposes, and
output projections - is queued and pumped into the kb-steps' idle slots.
Pairs are processed in an ACT-leveled order across q-tiles, and output
projections are deferred to the ACT-heaviest stretch. PSUM accumulation
exploits the HW behavior that a start=True matmul zeroes its whole bank.
"""
import sys
import os

for _p in ("/opt/trn_rl_repo", "/root/.axon_site/_ro/trn_rl_repo"):
    if os.path.isdir(_p) and _p not in sys.path:
        sys.path.insert(0, _p)

import numpy as np

import concourse.mybir as mybir
import concourse.tile as tile
from concourse import bacc
from concourse.bass_utils import run_bass_kernel_spmd

F32 = mybir.dt.float32
F32R = mybir.dt.float32r
BF16 = mybir.dt.bfloat16
I16 = mybir.dt.int16
AF = mybir.ActivationFunctionType
MULT = mybir.AluOpType.mult
ADD = mybir.AluOpType.add
DIV = mybir.AluOpType.divide

B, S, D = 2, 2048, 1024
H, DK = 16, 64
THETA = 10000.0
NCORES = 8
GROUPS = 4          # head groups (tensor parallel)
GH = H // GROUPS    # heads per group = 4
GF = GH * DK        # features per group = 256
SWAP_MASK = [i ^ 1 for i in range(32)]
TRICK_A = 0.125 * 1.4426950408889634 * 128.0
TRICK_B = 127.0 * 128.0 - 5.5
KVER = 52  # bump on any kernel change: busts the HLO-shape-keyed NEFF cache

_CACHED = {}


def _build_nc(iters=1):
    _iters = iters
    nc = bacc.Bacc("TRN2", target_bir_lowering=False, debug=False, num_devices=NCORES)
    # Host-prepacked inputs: partition-major [p, chunk, free] layouts so every
    # load is a straight slice with wide contiguous runs (fewer descriptors,
    # no small-transfer penalty, fewer DMAs on the serial HWDGE queue). wqk
    # chunk slots in first-use order [c0 (Q01), c2 (K01), c1 (Q23), c3 (K23)].
    xT = nc.dram_tensor("xTp", [128, 8, S], BF16, kind="ExternalInput").ap()
    wqkT = nc.dram_tensor("wqkp", [128, 8, 2 * GF], BF16, kind="ExternalInput").ap()
    wvT = nc.dram_tensor("wvp", [128, 8, GF], BF16, kind="ExternalInput").ap()
    woT = nc.dram_tensor("wop", [128, 2, D], BF16, kind="ExternalInput").ap()
    cosf = nc.dram_tensor("cosf", [128, S], BF16, kind="ExternalInput").ap()
    sins = nc.dram_tensor("sins", [128, S], BF16, kind="ExternalInput").ap()
    tri = nc.dram_tensor("tri", [128, 128], BF16, kind="ExternalInput").ap()
    ident = nc.dram_tensor("ident", [128, 128], BF16, kind="ExternalInput").ap()
    onesc = nc.dram_tensor("onesc", [128, 1], F32R, kind="ExternalInput").ap()
    # unused input whose shape encodes the kernel version: the neuron compile
    # cache keys on HLO structure only, so two kernels with identical I/O
    # shapes would otherwise collide.
    nc.dram_tensor("cachebust", [iters, KVER], F32, kind="ExternalInput")
    out = nc.dram_tensor("out", [S, D], BF16, kind="ExternalOutput").ap()

    SB = S // 512  # 4 q-tiles of 512
    KB = S // 128  # 16 k-blocks of 128
    SLOT = {0: 0, 2: 1, 1: 2, 3: 3}  # qk chunk -> wqk_sb column slot

    with tile.TileContext(nc) as tc:
        with tc.tile_pool(name="const", bufs=1) as cpool, \
             tc.tile_pool(name="big", bufs=1) as bpool, \
             tc.tile_pool(name="work", bufs=2) as wpool, \
             tc.tile_pool(name="asb", bufs=3) as apool, \
             tc.tile_pool(name="probs", bufs=4) as ppool, \
             tc.tile_pool(name="obuf", bufs=4) as opool, \
             tc.tile_pool(name="psum", bufs=1, space="PSUM") as psum:

            # ---- loads, ordered by first use on the single HWDGE queue ----
            wqk_sb = cpool.tile([128, 8, 2 * GF], BF16, tag="wqk")
            wv_sb = cpool.tile([128, 8, GF], BF16, tag="wv")
            wo_sb = cpool.tile([128, 2, D], BF16, tag="wo")
            cos_sb = cpool.tile([128, S], BF16, tag="cos")
            sin_sb = cpool.tile([128, S], BF16, tag="sin")
            tri_sb = cpool.tile([128, 128], BF16, tag="tri")
            ident_sb = cpool.tile([128, 128], BF16, tag="ident")
            onesc_sb = cpool.tile([128, 1], F32R, tag="onesc")
            xt_all = cpool.tile([128, 8, S], BF16, tag="xt")

            # critical-path first: deps of the first projection matmuls,
            # then the first pair's rope/attention constants, then the rest.
            nc.sync.dma_start(wqk_sb[:, 0:4, 0:128], wqkT[:, 0:4, 0:128])
            nc.sync.dma_start(xt_all[:, 0:2, 0:512], xT[:, 0:2, 0:512])
            nc.sync.dma_start(xt_all[:, 2:4, 0:512], xT[:, 2:4, 0:512])
            nc.sync.dma_start(wqk_sb[:, 4:8, 0:128], wqkT[:, 4:8, 0:128])
            nc.sync.dma_start(xt_all[:, 4:6, 0:512], xT[:, 4:6, 0:512])
            nc.sync.dma_start(xt_all[:, 6:8, 0:512], xT[:, 6:8, 0:512])
            nc.sync.dma_start(cos_sb[:, 0:512], cosf[:, 0:512])
            nc.sync.dma_start(sin_sb[:, 0:512], sins[:, 0:512])
            nc.sync.dma_start(wqk_sb[:, :, 128:256], wqkT[:, :, 128:256])
            nc.sync.dma_start(onesc_sb[:], onesc)
            nc.sync.dma_start(wv_sb[:], wvT)
            nc.sync.dma_start(tri_sb[:], tri)
            nc.sync.dma_start(wqk_sb[:, :, 256:512], wqkT[:, :, 256:512])
            nc.sync.dma_start(xt_all[:, :, 512:1024], xT[:, :, 512:1024])
            nc.sync.dma_start(cos_sb[:, 512:S], cosf[:, 512:S])
            nc.sync.dma_start(sin_sb[:, 512:S], sins[:, 512:S])
            nc.sync.dma_start(ident_sb[:], ident)
            nc.sync.dma_start(wo_sb[:], woT)
            nc.sync.dma_start(xt_all[:, :, 1024:1536], xT[:, :, 1024:1536])
            nc.sync.dma_start(xt_all[:, :, 1536:S], xT[:, :, 1536:S])

            warm = cpool.tile([1, 1], F32, tag="warm")
            nc.scalar.activation(warm[:], onesc_sb[0:1, 0:1], AF.Exp, scale=1.0)
            # Warm-up matmuls during the initial DMA wait: the PE runs at
            # half rate until it has been busy 3us, so burn the p-state ramp
            # on a zero row instead of the first real projections.
            zrow = cpool.tile([1, 512], BF16, tag="zrow")
            nc.vector.memset(zrow[:], 0.0)
            for w in range(5):
                pw = psum.tile([128, 2, 512], F32, tag="sc2", bufs=2,
                               name=f"warmmm{w}")
                nc.tensor.matmul(pw[:, 0, :], zrow[0:1, 0:128], zrow[0:1, :],
                                 start=True, stop=True)

            # ---- kernel body ----
            # Projections (phase 1) for x-tile t+1 are interleaved INTO the
            # attention kb-loop over qt=t: attention is ACT(exp)-bound, so the
            # PE fills its idle slots with the next tile's QK/V projections.
            for _it in range(iters):
              qkT = bpool.tile([128, 4, S], BF16, tag="qkT", name=f"qkT{_it}")
              vt = bpool.tile([128, KB, GH, DK + 1], BF16, tag="vt", name=f"vt{_it}")
              nc.vector.tensor_copy(
                  vt[:, :, :, DK:DK + 1],
                  onesc_sb[:, None, None, :].to_broadcast([128, KB, GH, 1]))

              def proj_qk_half(t, c, half, ps):
                  sl = SLOT[c]
                  for dc in range(4 * half, 4 * half + 4):
                      nc.tensor.matmul(
                          ps[:], wqk_sb[:, dc, sl * 128:(sl + 1) * 128],
                          xt_all[:, dc, t * 512:(t + 1) * 512],
                          start=(dc == 0), stop=(dc == 7))

              def proj_qk(t, c, fast=False, half=None):
                  # QK projection chunk: 128 features (head pair c of Q|K),
                  # 512 seq positions, full D contraction; then RoPE.
                  ps = psum.tile([128, 512], F32, tag="sc", bufs=2)
                  proj_qk_half(t, c, 0, ps)
                  if half is not None:
                      # second half (+ RoPE) deferred as the next filler unit
                      half.appendleft(lambda: proj_qk_rope(t, c, ps, fast))
                      return
                  proj_qk_rope(t, c, ps, fast)

              def proj_qk_rope(t, c, ps, fast=False):
                  proj_qk_half(t, c, 1, ps)
                  tsl = slice(t * 512, (t + 1) * 512)
                  # rope: qkT = ps*cos + swap(ps*sins), all on DVE (Pool is
                  # reserved for the latency-critical causal masks)
                  if fast:
                      # prologue chunks: pre-round ps to bf16 on the (idle)
                      # ACT engine so the DVE multiplies run in 2x mode
                      psb = wpool.tile([128, 512], BF16, tag="psb")
                      nc.scalar.copy(psb[:], ps[:])
                      src = psb
                  else:
                      src = ps
                  tmp = wpool.tile([128, 512], BF16, tag="ropetmp")
                  nc.vector.tensor_tensor(tmp[:], src[:], sin_sb[:, tsl], MULT)
                  tmp2 = wpool.tile([128, 512], BF16, tag="ropetmp2")
                  nc.vector.stream_shuffle(tmp2[:], tmp[:], SWAP_MASK)
                  nc.vector.tensor_tensor(qkT[:, c, tsl], src[:], cos_sb[:, tsl], MULT)
                  nc.gpsimd.tensor_tensor(qkT[:, c, tsl], qkT[:, c, tsl], tmp2[:], ADD)

              def proj_v(sb_i, on_act=False):
                  psv = psum.tile([128, GF], F32, tag="sc", bufs=2)
                  for dc in range(8):
                      nc.tensor.matmul(
                          psv[:], xt_all[:, dc, sb_i * 128:(sb_i + 1) * 128],
                          wv_sb[:, dc, :], start=(dc == 0), stop=(dc == 7))
                  if on_act:
                      nc.scalar.copy(vt[:, sb_i, :, 0:DK],
                                     psv[:].rearrange("p (h d) -> p h d", h=GH))
                  else:
                      nc.vector.tensor_copy(
                          vt[:, sb_i, :, 0:DK],
                          psv[:].rearrange("p (h d) -> p h d", h=GH))

              # Deferred-PE-work queue: projection chunks for tile t+1,
              # transposes of the previous pair, and the previous qt's output
              # projection all get pumped into the attention kb-loop so the
              # (in-order) PE never sits behind a dependency-stalled
              # instruction for long.
              from collections import deque
              fill_q = deque()   # prompt PE work (projections, transposes)
              late_q = deque()   # output projections, deferred to late qts
                                 # where attention has an ACT-vs-PE deficit

              def pump(n=1, late_ok=False):
                  for _ in range(n):
                      if fill_q:
                          fill_q.popleft()()
                      elif late_ok and late_q:
                          late_q.popleft()()

              def transpose_unit(gq, pair, att_sb, qb):
                  def run():
                      tps = psum.tile([128, 128], BF16, tag="sc", bufs=2,
                                      name=f"tps{gq}_{pair}_{_it}")
                      nc.tensor.transpose(tps[:], att_sb[:, qb, :], ident_sb[:])
                      nc.vector.tensor_copy(
                          attnT[pair][:, gq * 128:(gq + 1) * 128], tps[:])
                  return run

              def oproj_units(qb, split_copy=False):
                  # output projection for one 128-q-block, as two filler units
                  # (one per 512-wide n-half; the second issues the DMA)
                  st = {}

                  def run_nh(nh):
                      qsl = slice(qb * 128, (qb + 1) * 128)
                      if nh == 0:
                          st["osb"] = opool.tile([128, D], BF16, tag="osb",
                                                 name=f"osb{qb}_{_it}")
                      osb = st["osb"]
                      nsl = slice(nh * 512, (nh + 1) * 512)
                      pso = psum.tile([128, 512], F32, tag="sc", bufs=2)
                      nc.tensor.matmul(pso[:], attnT[0][:, qsl],
                                       wo_sb[:, 0, nsl], start=True, stop=False)
                      nc.tensor.matmul(pso[:], attnT[1][:, qsl],
                                       wo_sb[:, 1, nsl], start=False, stop=True)
                      if split_copy and nh == 1:
                          nc.scalar.copy(osb[:, nsl], pso[:])
                      else:
                          nc.vector.tensor_copy(osb[:, nsl], pso[:])
                      if split_copy:
                          # last qt: one DMA per n-half so the final transfer
                          # is half-size
                          nc.sync.dma_start(out[qsl, nsl], osb[:, nsl])
                      elif nh == 1:
                          nc.sync.dma_start(out[qsl, :], osb[:])

                  return [lambda: run_nh(0), lambda: run_nh(1)]

              # ---- attention (PV-flipped), deferred work interleaved ----
              # PV: attn[q, dk] = probsT.T @ [V|1] per 128-q-block: N=65 moving
              # rows instead of N=512, fully using the 128-wide K (k-positions)
              # and M (q) dims of the PE array. Softmax sums land in column 64
              # as per-partition scalars -> normalization via Pool broadcast
              # multiply, then a PE transpose restores [f, q] layout for the
              # output projection.
              attnT = [bpool.tile([128, S], BF16, tag=f"attnT{p}",
                                  name=f"attnT{p}_{_it}") for p in range(2)]

              # tile t=0: pair-0's needs (Q01, K01, V) up front; Q23/K23 queued.
              # fast=True / on_act=True shift prologue elementwise work onto
              # the idle ACT engine to shorten the first-attention latency.
              proj_qk(0, 0, fast=True)
              proj_qk(0, 2, fast=True)
              for s in range(4):
                  proj_v(s, on_act=True)
              fill_q.append(lambda: proj_qk(0, 1, fast=True))
              fill_q.append(lambda: proj_qk(0, 3, fast=True))

              def enqueue_proj(t):
                  for c in range(4):
                      fill_q.append(lambda c=c: proj_qk(t, c, fast=True, half=fill_q))
                  for s in range(4 * t, 4 * t + 4):
                      fill_q.append(lambda s=s: proj_v(s))

              # Pair schedule, leveled so the ACT-heavy qt=3 pairs sit next to
              # filler-rich regions instead of clustering at the end. proj[t]
              # is enqueued just before the first pair that leaves enough
              # steps to drain it ahead of its consumer.
              SCHED = [(0, 0, 1), (0, 1, None), (1, 0, 2), (1, 1, None),
                       (2, 0, 3), (3, 0, None), (2, 1, None), (3, 1, None)]
              done_pairs = set()

              def do_pair(qt, pair, late_ok):
                  nkb = 4 * qt + 4
                  nsteps = nkb
                  step = 0
                  qs, ks = pair, 2 + pair
                  last = qt == SB - 1 and pair == 1
                  # Interleaved accumulation GROUPS in one PSUM bank are
                  # broken on HW: start=True zeroes the WHOLE bank (HW
                  # verified). Exploit that: the first PV write of each
                  # h-bank (kb=0, qb=0) runs with start=True to zero the
                  # bank, everything else accumulates with start=False.
                  attps = [psum.tile([128, 4, DK + 1], F32, tag="att",
                                     bufs=2, name=f"attps{h}_{qt}_{pair}_{_it}")
                           for h in range(2)]

                  def pv(kb):
                      for qb in range(max(kb - 4 * qt, 0), 4):
                          gq = 4 * qt + qb
                          for h in range(2):
                              nc.tensor.matmul(
                                  attps[h][:, qb, :],
                                  pab_ring[kb % 32][:, h, qb * 128:(qb + 1) * 128],
                                  vt[:, kb, 2 * pair + h, :],
                                  start=(kb == 0 and qb == 0),
                                  stop=(kb == gq),
                                  skip_group_check=True)

                  att_sb = apool.tile([128, 4, 128], BF16, tag="attsb")
                  rsum = wpool.tile([128, 4, 2], F32, tag="rsum")

                  def normalize(qb):
                      # DVE reciprocal of the col-64 sums + DVE broadcast
                      # multiply (gpsimd cannot read PSUM). Per-q-block only
                      # for the very last pair (to pipeline the tail);
                      # batched per-pair otherwise to keep the DVE
                      # instruction count down. Transposes go to the FRONT of
                      # the queue: they are small, release the shared "sc"
                      # PSUM ring fast, and feed the output projection.
                      if not last:
                          if qb < 3:
                              return
                          qsl3, nq = slice(0, 4), 4
                      else:
                          qsl3, nq = slice(qb, qb + 1), 1
                      for h in range(2):
                          nc.vector.reciprocal(rsum[:, qsl3, h:h + 1],
                                               attps[h][:, qsl3, DK:DK + 1])
                          nc.vector.tensor_tensor(
                              att_sb[:, qsl3, h * 64:(h + 1) * 64],
                              attps[h][:, qsl3, 0:DK],
                              rsum[:, qsl3, h:h + 1].to_broadcast(
                                  [128, nq, DK]), MULT)
                      if last:
                          # last pair: output projection chases each q-block
                          # (PE transpose: shorter latency than the DMA XBAR)
                          for u in reversed(oproj_units(4 * qt + qb,
                                                        split_copy=True)):
                              fill_q.appendleft(u)
                          fill_q.appendleft(
                              transpose_unit(4 * qt + qb, pair, att_sb, qb))
                      else:
                          # engine-free transpose via the DMA XBAR: frees PE
                          # rows, the DVE copy, and the "sc" PSUM ring
                          for b in range(4):
                              gq = 4 * qt + b
                              nc.sync.dma_start_transpose(
                                  attnT[pair][:, gq * 128:(gq + 1) * 128],
                                  att_sb[:, b, :])

                  pab_ring = {}
                  for kb in range(nkb):
                      lam = max(kb - 4 * qt, 0) * 128
                      qsl = slice(qt * 512 + lam, (qt + 1) * 512)
                      ksl = slice(kb * 128, (kb + 1) * 128)
                      ss = psum.tile([128, 2, 512], F32, tag="sc2", bufs=2)
                      nc.tensor.matmul(ss[:, 0, lam:512], qkT[0:64, ks, ksl],
                                       qkT[0:64, qs, qsl], start=True, stop=True)
                      nc.tensor.matmul(ss[:, 1, lam:512], qkT[64:128, ks, ksl],
                                       qkT[64:128, qs, qsl], start=True, stop=True)
                      pab = ppool.tile([128, 2, 512], BF16, tag="probs", bufs=32)
                      pab_ring[kb % 32] = pab
                      nc.scalar.activation(pab[:, :, lam:512], ss[:, :, lam:512], AF.Exp, scale=0.125)
                      if kb >= 4 * qt:  # diagonal block: causal tri mask
                          # on DVE: all-bf16 SBUF operands hit 2x mode,
                          # shortening the exp->mask->PV diagonal chain
                          dsl = slice(lam, lam + 128)
                          nc.vector.tensor_tensor(
                              pab[:, :, dsl], pab[:, :, dsl],
                              tri_sb[:, None, :].to_broadcast([128, 2, 128]), MULT)
                      # drain queued PE work evenly across this pair's steps;
                      # late (output-projection) work backfills ACT-bound pairs
                      n = -(-len(fill_q) // (nsteps - step)) if fill_q else 0
                      if kb >= 4 * qt and late_ok:
                          n = max(n, 2)  # diag steps absorb more filler
                      if late_ok and (step % 2 == 0 or kb >= 4 * qt):
                          n = max(n, 1)
                      pump(n, late_ok=late_ok)
                      step += 1
                      # software pipeline: PV for the previous kb runs after
                      # this kb's scores are already in flight
                      if kb > 0:
                          pv(kb - 1)
                          if kb - 1 >= 4 * qt:  # that region just stopped
                              normalize(kb - 1 - 4 * qt)
                  pv(nkb - 1)
                  normalize(3)
                  done_pairs.add((qt, pair))
                  if (qt, 1 - pair) in done_pairs and not last:
                      for qb in range(4 * qt, 4 * qt + 4):
                          late_q.extend(oproj_units(qb))

              for i, (qt, pair, tload) in enumerate(SCHED):
                  if tload is not None:
                      enqueue_proj(tload)
                  do_pair(qt, pair, late_ok=(i >= len(SCHED) - 3))
              while late_q:
                  late_q.popleft()()
              while fill_q:
                  fill_q.popleft()()

    nc.compile()
    return nc


def _host_tables(token_positions):
    pos = np.asarray(token_positions, dtype=np.float32)  # [S]
    half = DK // 2
    freq = THETA ** (-np.arange(0, DK, 2, dtype=np.float32) / DK)  # [32]
    # per-partition tables on [dk(128 = 2 heads of 64), s]
    f64 = np.repeat(freq, 2)          # [64] freq per feature index
    ang64 = pos[None, :] * f64[:, None]  # [64, S]
    cos64 = np.cos(ang64)
    sin64 = np.sin(ang64)
    sign = np.where(np.arange(DK) % 2 == 0, 1.0, -1.0).astype(np.float32)  # +s even, -s odd
    sins64 = sin64 * sign[:, None]
    from ml_dtypes import bfloat16 as bf16
    cosf = np.concatenate([cos64, cos64], axis=0).astype(bf16)   # [128, S]
    sins = np.concatenate([sins64, sins64], axis=0).astype(bf16)  # [128, S]
    return cosf, sins


def kernel(x, Wq, Wk, Wv, Wo, token_positions):
    from ml_dtypes import bfloat16 as bf16
    x = np.asarray(x, dtype=np.float32)
    Wq = np.asarray(Wq, dtype=np.float32)
    Wk = np.asarray(Wk, dtype=np.float32)
    Wv = np.asarray(Wv, dtype=np.float32)
    Wo = np.asarray(Wo, dtype=np.float32)

    if "nc" not in _CACHED:
        _CACHED["nc"] = _build_nc(iters=int(os.environ.get("BENCH_ITERS", "1")))
    nc = _CACHED["nc"]

    cosf, sins = _host_tables(token_positions)
    tri = np.triu(np.ones((128, 128), dtype=bf16))  # tri[k, j] = 1 if j >= k
    ident = np.eye(128, dtype=bf16)
    onesc = np.ones((128, 1), dtype=np.float32)

    def pack(mT, nchunk):
        n = mT.shape[1]
        return np.ascontiguousarray(
            np.asarray(mT, dtype=np.float32).reshape(nchunk, 128, n)
            .transpose(1, 0, 2)).astype(bf16)

    xTp = [pack(x[b].T, 8) for b in range(B)]
    in_maps = []
    for c in range(NCORES):
        b, g = c // GROUPS, c % GROUPS
        R = slice(g * GF, (g + 1) * GF)
        wqkT = np.concatenate([Wq[R].T, Wk[R].T], axis=1)  # [D, 512]
        # chunk slots in first-use order: c0 (Q01), c2 (K01), c1 (Q23), c3
        wqkT = wqkT[:, [*range(0, 128), *range(256, 384),
                        *range(128, 256), *range(384, 512)]]
        in_maps.append({
            "xTp": xTp[b], "wqkp": pack(wqkT, 8), "wvp": pack(Wv[R].T, 8),
            "wop": pack(Wo[:, R].T, 2),
            "cosf": cosf, "sins": sins, "tri": tri, "ident": ident, "onesc": onesc,
            "cachebust": np.zeros((int(os.environ.get("BENCH_ITERS", "1")), KVER), dtype=np.float32),
        })

    try:
        res = run_bass_kernel_spmd(nc, in_maps, core_ids=list(range(NCORES)))
    except Exception:
        # transient NRT_EXEC_UNIT_UNRECOVERABLE flakes recover on retry
        import time as _time
        _time.sleep(2.0)
        res = run_bass_kernel_spmd(nc, in_maps, core_ids=list(range(NCORES)))
    _CACHED["last_results"] = res
    outs = [np.asarray(r["out"], dtype=np.float32) for r in res.results]  # [S, D]
    full = np.empty((B, S, D), dtype=np.float32)
    for b in range(B):
        full[b] = sum(outs[b * GROUPS + g] for g in range(GROUPS))
    return full

